# revision 48
# baseline (speedup 1.0000x reference)
"""NlmCNN (weight-predicting CNN + per-pixel 13x13 weighted sum) on 8 trn2 cores.

Sharding: data-parallel over batch (8 images -> 8 cores), weights replicated.

Per-core layout trick: output y is the conv stack's result center-cropped by
6 pixels, and the receptive field of the three 3x3 convs only reaches 3 px
out, so SAME-padding never materializes: every layer is computed VALID-style
on an unpadded 256-stride flat layout. Column-wrap junk from flat shifted
reads stays confined to the outer <=3 columns of each layer, which are
discarded by the crop.

Pipeline per strip of S output rows (strips software-pipelined: conv1 of
strip i+1 is emitted during strip i's conv3 phase):
  conv1: per-2-chunk im2col [9, 1024] via one 3-dim DMA -> K=9 matmul
  conv2/conv3: taps (du,0)+(du,1) fused into K=128 matmuls against an SBUF
         tile whose upper 64 partitions hold h shifted by +1 (built by two
         SBUF->SBUF SWDGE DMAs per strip); taps (du,2) are K=64 singles
  einsum: patch matrix xs[c, s] = x[pos + shift(c)] via one contiguous DMA
         per tap-row u (xs and t keep a 256-spaced layout so the gather is
         13 fat descriptors, not 208 thin ones); DVE scalar_tensor_tensor
         computes t = (conv3_psum + b3) * xs from PSUM; "staircase ones"
         matmuls (deferred so the PE never waits on the DVE) reduce over
         partitions, accumulating 2-row chunk j into row j of one
         persistent PSUM tile; one copy + one DMA store the whole image.

All matmul operands are bf16 (weights quantized once at setup; activations
written bf16 by the relu): stationary loads stream at 1 col/cycle vs
fp32r's ~2.5, and h/imc DMA bytes halve. xs stays fp32 for accuracy.
"""

import numpy as np

import concourse.bacc as bacc
import concourse.bass as bass
import concourse.mybir as mybir
import concourse.tile as tile
from concourse.bass_utils import run_bass_kernel_spmd

F32 = mybir.dt.float32
F32R = mybir.dt.float32r
BF16 = mybir.dt.bfloat16
AF = mybir.ActivationFunctionType
ALU = mybir.AluOpType

H = 256
W = 256
K = 13
HO = H - K + 1  # 244
CH = 64
C3 = K * K  # 169
C3L = 9 * K  # 117 channels: u in [0,9)
C3H = 4 * K  # 52 channels:  u in [9,13)
S_STRIP = 16
NC_ = 512  # chunk positions (2 image rows)
import os
PIPE = os.environ.get("K_PIPE", "1") == "1"      # cross-strip sw pipelining
XS_GP = os.environ.get("K_XS_GP", "1") == "1"    # xs DMAs on gpsimd queue
DUP_GP = os.environ.get("K_DUP_GP", "1") == "1"  # dup DMAs on gpsimd queue


def _ap(t, off, dims):
    return bass.AP(t, off, [list(d) for d in dims])


def _mm(nc, out, lhsT, rhs, start, stop):
    nc.tensor.matmul(out, lhsT, rhs, start=start, stop=stop)


def build_nc():
    nc = bacc.Bacc("TRN2", target_bir_lowering=False, debug=False)

    x = nc.dram_tensor("x", [1, 1, H, W], F32, kind="ExternalInput")
    w1 = nc.dram_tensor("W1", [CH, 1, 3, 3], F32, kind="ExternalInput")
    b1 = nc.dram_tensor("b1", [CH], F32, kind="ExternalInput")
    w2 = nc.dram_tensor("W2", [CH, CH, 3, 3], F32, kind="ExternalInput")
    b2 = nc.dram_tensor("b2", [CH], F32, kind="ExternalInput")
    w3 = nc.dram_tensor("W3", [C3, CH, 3, 3], F32, kind="ExternalInput")
    b3 = nc.dram_tensor("b3", [C3], F32, kind="ExternalInput")
    y = nc.dram_tensor("y", [1, 1, HO, HO], F32, kind="ExternalOutput")
    xr = nc.dram_tensor("x_r", [H * W], F32R)
    xb = nc.dram_tensor("x_b", [H * W], BF16)

    with tile.TileContext(nc) as tc:
        Body(nc, tc, x, w1, b1, w2, b2, w3, b3, y, xr, xb).build()

    nc.compile()
    return nc


class Body:
    def __init__(self, nc, tc, x, w1, b1, w2, b2, w3, b3, y, xr, xb):
        self.nc, self.tc = nc, tc
        self.x, self.w1, self.b1, self.w2, self.b2 = x, w1, b1, w2, b2
        self.w3, self.b3, self.y, self.xr, self.xb = w3, b3, y, xr, xb

    def build(self):
        nc, tc = self.nc, self.tc
        with (
            tc.tile_pool(name="consts", bufs=1) as consts,
            tc.tile_pool(name="tlo", bufs=1) as p_tlo,
            tc.tile_pool(name="thi", bufs=1) as p_thi,
            tc.tile_pool(name="imc", bufs=4) as p_imc,
            tc.tile_pool(name="h1p", bufs=2) as p_h1,
            tc.tile_pool(name="h2p", bufs=1) as p_h2,
            tc.tile_pool(name="h2q", bufs=1) as p_h2b,
            tc.tile_pool(name="xsl", bufs=2) as p_xsl,
            tc.tile_pool(name="xsh", bufs=2) as p_xsh,
            tc.tile_pool(name="yout", bufs=1) as p_y,
            tc.tile_pool(name="ps12", bufs=2, space="PSUM") as ps12,
            tc.tile_pool(name="ps3", bufs=2, space="PSUM") as ps3,
            tc.tile_pool(name="psy", bufs=1, space="PSUM") as psy,
        ):
            self.consts = consts
            self.p_tlo, self.p_thi, self.p_imc = p_tlo, p_thi, p_imc
            self.p_h1, self.p_h2, self.p_xsl, self.p_xsh = p_h1, p_h2, p_xsl, p_xsh
            self.p_h2b = p_h2b
            self.p_y, self.ps12, self.ps3, self.psy = p_y, ps12, ps3, psy
            self._build_consts()
            self._build_strips()

    def _build_consts(self):
        nc, tc, consts = self.nc, self.tc, self.consts
        stage = tc.alloc_tile_pool(name="stage", bufs=1)
        pwtr = tc.alloc_tile_pool(name="wtr", bufs=1, space="PSUM")

        # Weights arrive [co, ci, du, dv]; matmuls need [ci, co] per tap.
        # A strided gather DMA would be 4-byte-descriptor-bound, so load
        # contiguously and transpose on the PE instead.
        from concourse.masks import make_identity

        ident = stage.tile([128, 128], F32)
        make_identity(nc, ident[:])

        w1raw = stage.tile([CH, 9], F32)
        nc.sync.dma_start(out=w1raw[:], in_=_ap(self.w1, 0, [(9, CH), (1, 9)]))
        w2raw = stage.tile([CH, 9 * CH], F32)
        nc.sync.dma_start(out=w2raw[:], in_=_ap(self.w2, 0, [(9 * CH, CH), (1, 9 * CH)]))
        w3raw_a = stage.tile([128, 9 * CH], F32)
        nc.sync.dma_start(
            out=w3raw_a[:], in_=_ap(self.w3, 0, [(9 * CH, 128), (1, 9 * CH)])
        )
        w3raw_b = stage.tile([C3 - 128, 9 * CH], F32)
        nc.sync.dma_start(
            out=w3raw_b[:],
            in_=_ap(self.w3, 128 * 9 * CH, [(9 * CH, C3 - 128), (1, 9 * CH)]),
        )

        def tapv(raw, t, n):  # [n_co, ci] view of tap t
            return raw[0:n, :].rearrange("p (ci t) -> p t ci", t=9)[:, t, :]

        # w1: lhsT [9 taps, 64 co]; copy at partitions 64-72 for the
        # row-tiled chunk-b matmul (lhs/rhs start partitions must match)
        pT = pwtr.tile([128, 128], F32, tag="wtr")
        nc.tensor.transpose(pT[0:9, 0:CH], w1raw[:], ident[0:CH, 0:CH])
        self.w1sb = consts.tile([128, CH], F32R)
        nc.vector.tensor_copy(self.w1sb[0:9, :], pT[0:9, 0:CH])
        nc.sync.dma_start(out=self.w1sb[64:73, :], in_=self.w1sb[0:9, :])

        # Transpose each tap to PSUM base 0 (HW requires base 0), cast to
        # bf16; upper (tap (du,1)) halves staged then partition-shifted to
        # partitions 64-127 by one SBUF->SBUF DMA per weight tile.
        NB = C3 - 128  # 41
        self.w2p = consts.tile([2 * CH, 3 * CH], F32R)
        self.w2s = consts.tile([CH, 3 * CH], F32R)
        self.w3p_lo = consts.tile([2 * CH, 3 * C3L], F32R)
        self.w3p_hi = consts.tile([2 * CH, 3 * C3H], F32R)
        # q-pair: taps (0,2)+(1,2) fused against the shift-2/shift-(W+2)
        # dup tile h2b; only tap (2,2) remains a K=64 single
        self.w3q_lo = consts.tile([2 * CH, C3L], F32R)
        self.w3q_hi = consts.tile([2 * CH, C3H], F32R)
        self.w3s_lo = consts.tile([CH, C3L], F32R)
        self.w3s_hi = consts.tile([CH, C3H], F32R)
        w2pu = stage.tile([CH, 3 * CH], F32R)
        w3pu_lo = stage.tile([CH, 3 * C3L], F32R)
        w3pu_hi = stage.tile([CH, 3 * C3H], F32R)
        w3qu_lo = stage.tile([CH, C3L], F32R)
        w3qu_hi = stage.tile([CH, C3H], F32R)

        def tr(dst, raw, t, n, c0_, c1_):
            pT = pwtr.tile([CH, 128], F32, tag="wtr")
            nc.tensor.transpose(pT[:, 0:n], tapv(raw, t, n), ident[0:n, 0:n])
            nc.vector.tensor_copy(dst, pT[:, c0_:c1_])

        for p in range(3):
            cw = slice(p * CH, (p + 1) * CH)
            cl = slice(p * C3L, (p + 1) * C3L)
            ch0 = slice(p * C3H, p * C3H + 11)
            ch1 = slice(p * C3H + 11, (p + 1) * C3H)
            tr(self.w2p[0:CH, cw], w2raw, p * 3, CH, 0, CH)
            tr(w2pu[:, cw], w2raw, p * 3 + 1, CH, 0, CH)
            tr(self.w2s[:, cw], w2raw, p * 3 + 2, CH, 0, CH)
            pT = pwtr.tile([CH, 128], F32, tag="wtr")
            nc.tensor.transpose(pT[:], tapv(w3raw_a, p * 3, 128), ident[:])
            nc.vector.tensor_copy(self.w3p_lo[0:CH, cl], pT[:, 0:C3L])
            nc.vector.tensor_copy(self.w3p_hi[0:CH, ch0], pT[:, C3L:128])
            pT = pwtr.tile([CH, 128], F32, tag="wtr")
            nc.tensor.transpose(pT[:], tapv(w3raw_a, p * 3 + 1, 128), ident[:])
            nc.vector.tensor_copy(w3pu_lo[:, cl], pT[:, 0:C3L])
            nc.vector.tensor_copy(w3pu_hi[:, ch0], pT[:, C3L:128])
            # tap (p,2): p=0 -> q lower, p=1 -> q upper (staged), p=2 -> single
            s_lo, s_hi = {
                0: (self.w3q_lo[0:CH, :], self.w3q_hi[0:CH, :]),
                1: (w3qu_lo[:], w3qu_hi[:]),
                2: (self.w3s_lo[:], self.w3s_hi[:]),
            }[p]
            pT = pwtr.tile([CH, 128], F32, tag="wtr")
            nc.tensor.transpose(pT[:], tapv(w3raw_a, p * 3 + 2, 128), ident[:])
            nc.vector.tensor_copy(s_lo, pT[:, 0:C3L])
            nc.vector.tensor_copy(s_hi[:, 0:11], pT[:, C3L:128])
            tr(self.w3p_hi[0:CH, ch1], w3raw_b, p * 3, NB, 0, NB)
            tr(w3pu_hi[:, ch1], w3raw_b, p * 3 + 1, NB, 0, NB)
            tr(s_hi[:, 11:C3H], w3raw_b, p * 3 + 2, NB, 0, NB)
        nc.sync.dma_start(out=self.w2p[CH:, :], in_=w2pu[:])
        nc.sync.dma_start(out=self.w3p_lo[CH:, :], in_=w3pu_lo[:])
        nc.sync.dma_start(out=self.w3p_hi[CH:, :], in_=w3pu_hi[:])
        nc.sync.dma_start(out=self.w3q_lo[CH:, :], in_=w3qu_lo[:])
        nc.sync.dma_start(out=self.w3q_hi[CH:, :], in_=w3qu_hi[:])

        # biases replicated into partitions 64-127 for the chunk-b relus
        # (engine lanes are partition-hardwired)
        self.b1sb = consts.tile([2 * CH, 1], F32)
        nc.scalar.dma_start(out=self.b1sb[0:CH], in_=_ap(self.b1, 0, [(1, CH), (0, 1)]))
        nc.scalar.dma_start(out=self.b1sb[CH:], in_=_ap(self.b1, 0, [(1, CH), (0, 1)]))
        self.b2sb = consts.tile([2 * CH, 1], F32)
        nc.scalar.dma_start(out=self.b2sb[0:CH], in_=_ap(self.b2, 0, [(1, CH), (0, 1)]))
        nc.scalar.dma_start(out=self.b2sb[CH:], in_=_ap(self.b2, 0, [(1, CH), (0, 1)]))
        self.b3lo = consts.tile([C3L, 1], F32)
        nc.scalar.dma_start(out=self.b3lo[:], in_=_ap(self.b3, 0, [(1, C3L), (0, 1)]))
        self.b3hi = consts.tile([C3H, 1], F32)
        nc.scalar.dma_start(out=self.b3hi[:], in_=_ap(self.b3, C3L, [(1, C3H), (0, 1)]))

        # staircase-ones: stair[:, 128] = 1, else 0; column j of the view
        # stair[:, 128-j : 256-j] is all-ones -> matmul writes the partition
        # sum into PSUM row j (zeros elsewhere, harmless under accumulation)
        stair_st = stage.tile([C3L, 256], F32)
        nc.vector.memset(stair_st[:], 0.0)
        nc.vector.memset(stair_st[:, 128:129], 1.0)
        self.stair_lo = consts.tile([C3L, 256], F32R)
        nc.vector.tensor_copy(self.stair_lo[:], stair_st[:])
        self.stair_hi = self.stair_lo[0:C3H, :]

        # bf16 zeros for h-tile junk tails
        self.zs = consts.tile([2 * CH, 772], F32R)
        nc.vector.memset(self.zs[:].bitcast(F32), 0.0)

        # x -> fp32r copy in DRAM (conv1 im2col source) and bf16 copy (xs
        # gather source: xs feeds only the DVE, so half-width is fine)
        xst = stage.tile([128, H * W // 128], F32)
        nc.sync.dma_start(
            out=xst[:], in_=_ap(self.x, 0, [(H * W // 128, 128), (1, H * W // 128)])
        )
        xsr = stage.tile([128, H * W // 128], F32R)
        nc.vector.tensor_copy(xsr[:], xst[:])
        nc.sync.dma_start(
            out=_ap(self.xr, 0, [(H * W // 128, 128), (1, H * W // 128)]), in_=xsr[:]
        )
        xsb = stage.tile([128, H * W // 128], BF16)
        nc.vector.tensor_copy(xsb[:], xst[:])
        nc.sync.dma_start(
            out=_ap(self.xb, 0, [(H * W // 128, 128), (1, H * W // 128)]), in_=xsb[:]
        )
        pwtr.release()
        stage.release()

    # ---------------- per-strip stages ----------------

    def emit_conv1(self, i0, S):
        # Chunk-paired via PE array tiling: chunk a (first half-strip) runs
        # in tile (0,0) [SBUF 0-31 -> PSUM 0-63], chunk b (second half) in
        # tile (64,64) [SBUF 64-95 -> PSUM 64-127], concurrently. relu-a
        # writes h lower; relu-b (lanes 64-127) writes h upper pre-shifted;
        # two coarse dup DMAs fill in the opposite halves.
        nc = self.nc
        c0 = i0 + 6
        L1 = (S + 6) * W
        h1t = self.p_h1.tile([2 * CH, (S_STRIP + 6) * W + 772], F32R, tag="h1")
        nc.vector.tensor_copy(h1t[0:CH, L1 : L1 + 772], self.zs[0:CH])
        nc.vector.tensor_copy(h1t[CH:, L1 - 1 : L1 + 771], self.zs[CH:])
        Lh = (L1 // (2 * NC_)) * NC_
        for hs in range(0, L1, 2 * NC_):
            he = min(hs + 2 * NC_, L1)
            imc = self.p_imc.tile([9, 2 * NC_], F32R, tag="imc")
            nc.sync.dma_start(
                out=imc[:, 0 : he - hs],
                in_=_ap(self.xr, (c0 - 5) * W - 1 + hs, [(W, 3), (1, 3), (1, he - hs)]),
            )
            for cs in range(hs, he, NC_):
                ce = min(cs + NC_, L1)
                pt = self.ps12.tile([CH, NC_], F32, tag="ps12")
                _mm(nc, pt[:, 0 : ce - cs], self.w1sb[0:9, :],
                    imc[:, cs - hs : ce - hs], True, True)
                nc.scalar.activation(
                    h1t[0:CH, cs:ce], pt[:, 0 : ce - cs], AF.Relu,
                    bias=self.b1sb[0:CH],
                )
                dup = nc.gpsimd if DUP_GP else nc.sync
                if ce == Lh:
                    dup.dma_start(out=h1t[CH:, 0 : Lh - 1], in_=h1t[0:CH, 1:Lh])
                elif ce == L1:
                    dup.dma_start(
                        out=h1t[CH:, Lh - 1 : L1 - 1], in_=h1t[0:CH, Lh:L1]
                    )
        return h1t

    def emit_xs(self, i0, S):
        # xs[(u,v), i*W + j] = x[i0+u+i, j+v]: one contiguous fp32 read per
        # tap-row u (13 partitions x (S-1)*W+244 elements) into the spaced
        # layout; cols 244..256 of each row hold neighbor-row junk that the
        # stt views never touch.
        nc = self.nc
        LS = (S - 1) * W + HO
        xs_lo = self.p_xsl.tile([C3L, S_STRIP * W], BF16, tag="xsl")
        eng_lo = nc.gpsimd if XS_GP else nc.scalar
        eng_hi = nc.gpsimd if XS_GP else nc.sync
        for u in range(9):
            eng_lo.dma_start(
                out=xs_lo[u * K : (u + 1) * K, 0:LS],
                in_=_ap(self.xb, (i0 + u) * W, [(1, K), (1, LS)]),
            )
        xs_hi = self.p_xsh.tile([C3H, S_STRIP * W], BF16, tag="xsh")
        for u in range(4):
            eng_hi.dma_start(
                out=xs_hi[u * K : (u + 1) * K, 0:LS],
                in_=_ap(self.xb, (i0 + 9 + u) * W, [(1, K), (1, LS)]),
            )
        return xs_lo, xs_hi

    def emit_conv2(self, i0, S, h1t):
        # Same chunk-pairing as conv1, but K=128 pair taps: tile (0,0)
        # [128x64 T0] and (0,64) [128x64 T1]; K=64 singles use 64x64 tiles
        # (0,0)/(0,64).
        nc = self.nc
        L2 = (S + 3) * W
        h2t = self.p_h2.tile([2 * CH, (S_STRIP + 3) * W + 772], F32R, tag="h2")
        nc.vector.tensor_copy(h2t[0:CH, L2 : L2 + 772], self.zs[0:CH])
        nc.vector.tensor_copy(h2t[CH:, L2 - 1 : L2 + 771], self.zs[CH:])
        Lh = (L2 // (2 * NC_)) * NC_
        for cs in range(0, L2, NC_):
            ce = min(cs + NC_, L2)
            pt = self.ps12.tile([CH, NC_], F32, tag="ps12")
            for p in range(3):
                off = p * W + 255
                _mm(nc, pt[:, 0 : ce - cs], self.w2p[:, p * CH : (p + 1) * CH],
                    h1t[:, cs + off : ce + off], p == 0, False)
            for p in range(3):
                off = p * W + 2 + 255
                _mm(nc, pt[:, 0 : ce - cs], self.w2s[:, p * CH : (p + 1) * CH],
                    h1t[0:CH, cs + off : ce + off], False, p == 2)
            nc.scalar.activation(
                h2t[0:CH, cs:ce], pt[:, 0 : ce - cs], AF.Relu, bias=self.b2sb[0:CH]
            )
            dup = nc.gpsimd if DUP_GP else nc.sync
            if ce == Lh:
                dup.dma_start(out=h2t[CH:, 0 : Lh - 1], in_=h2t[0:CH, 1:Lh])
            elif ce == L2:
                dup.dma_start(out=h2t[CH:, Lh - 1 : L2 - 1], in_=h2t[0:CH, Lh:L2])
        # second dup tile for the q-pair: lower = h2<<2, upper = h2<<(W+2)
        h2b = self.p_h2b.tile([2 * CH, (S_STRIP + 3) * W + 772], F32R, tag="h2b")
        dup = nc.gpsimd if DUP_GP else nc.sync
        dup.dma_start(out=h2b[0:CH, 0 : Lh - 2], in_=h2t[0:CH, 2:Lh])
        dup.dma_start(out=h2b[0:CH, Lh - 2 : L2 - 2], in_=h2t[0:CH, Lh:L2])
        dup.dma_start(out=h2b[CH:, 0 : Lh - W - 2], in_=h2t[0:CH, W + 2 : Lh])
        dup.dma_start(out=h2b[CH:, Lh - W - 2 : L2 - W - 2], in_=h2t[0:CH, Lh:L2])
        return h2t, h2b

    def emit_conv3_chunk(self, i0, cs, h2t, h2b, xs_lo, xs_hi, t_lo, t_hi):
        """conv3 + stt for one 2-row chunk; staircase matmuls are deferred."""
        nc = self.nc
        plo = self.ps3.tile([C3L, NC_], F32, tag="ps3lo")
        phi = self.ps3.tile([C3H, NC_], F32, tag="ps3hi")
        for p in range(3):
            off = p * W + 255
            _mm(nc, plo[:], self.w3p_lo[:, p * C3L : (p + 1) * C3L],
                h2t[:, cs + off : cs + NC_ + off], p == 0, False)
            _mm(nc, phi[:], self.w3p_hi[:, p * C3H : (p + 1) * C3H],
                h2t[:, cs + off : cs + NC_ + off], p == 0, False)
        _mm(nc, plo[:], self.w3q_lo[:], h2b[:, cs + 255 : cs + NC_ + 255],
            False, False)
        _mm(nc, phi[:], self.w3q_hi[:], h2b[:, cs + 255 : cs + NC_ + 255],
            False, False)
        off = 2 * W + 2 + 255
        _mm(nc, plo[:], self.w3s_lo[:], h2t[0:CH, cs + off : cs + NC_ + off],
            False, True)
        _mm(nc, phi[:], self.w3s_hi[:], h2t[0:CH, cs + off : cs + NC_ + off],
            False, True)
        r2 = cs // W
        jj = (i0 + r2) // 2
        # t = (conv3_psum + b3) * xs, straight from PSUM on the DVE; all
        # three operands live in the 256-spaced [c, (r, col)] layout
        for ps_t, xs_t, t_t, b3_t, stair, kk in (
            (plo, xs_lo, t_lo, self.b3lo, self.stair_lo, C3L),
            (phi, xs_hi, t_hi, self.b3hi, self.stair_hi, C3H),
        ):
            wv = ps_t[:].rearrange("p (r c) -> p r c", c=W)[:, :, 6 : 6 + HO]
            xv = xs_t[:, cs : cs + NC_].rearrange("p (r c) -> p r c", c=W)[:, :, 0:HO]
            tv = t_t[:, cs : cs + NC_].rearrange("p (r c) -> p r c", c=W)[:, :, 0:HO]
            nc.vector.scalar_tensor_tensor(
                out=tv, in0=wv, scalar=b3_t[:], in1=xv, op0=ALU.add, op1=ALU.mult
            )
            self.pend.append((stair, t_t, r2, jj, kk))

    def flush_stair(self, keep=0):
        nc = self.nc
        while len(self.pend) > keep:
            stair, t_t, r2, jj, kk = self.pend.pop(0)
            stop = jj == self.NYC - 1 and kk == C3H
            rhs = t_t[:, r2 * W : (r2 + 2) * W].rearrange(
                "p (r c) -> p r c", c=W)[:, :, 0:HO]
            _mm(nc, self.psum_y[:], stair[:, 128 - jj : 256 - jj],
                rhs, self.first_mm, stop)
            self.first_mm = False

    def _build_strips(self):
        nc = self.nc
        self.NYC = (HO * HO) // 488  # 122
        self.psum_y = self.psy.tile([128, 488], F32)
        self.pend = []
        self.first_mm = True

        strips = []
        i0 = 0
        while i0 < HO:
            strips.append((i0, min(S_STRIP, HO - i0)))
            i0 += S_STRIP

        h1t = self.emit_conv1(*strips[0])
        xs = self.emit_xs(*strips[0])
        for si, (i0, S) in enumerate(strips):
            h2t, h2b = self.emit_conv2(i0, S, h1t)
            t_lo = self.p_tlo.tile([C3L, S_STRIP * W], F32R, tag="tlo")
            t_hi = self.p_thi.tile([C3H, S_STRIP * W], F32R, tag="thi")
            xs_lo, xs_hi = xs
            # prefetch next strip's xs while this strip's conv3 runs
            if si + 1 < len(strips):
                xs = self.emit_xs(*strips[si + 1])
            for ci, cs in enumerate(range(0, S * W, NC_)):
                self.emit_conv3_chunk(i0, cs, h2t, h2b, xs_lo, xs_hi, t_lo, t_hi)
                self.flush_stair(keep=4 if PIPE else 0)
                # overlap next strip's conv1 with this strip's conv3 tail
                if PIPE and ci == 1 and si + 1 < len(strips):
                    h1t = self.emit_conv1(*strips[si + 1])
            if not PIPE and si + 1 < len(strips):
                h1t = self.emit_conv1(*strips[si + 1])
            self.flush_stair(keep=0)

        ysb = self.p_y.tile([self.NYC, 488], F32)
        nc.vector.tensor_copy(ysb[:], self.psum_y[0 : self.NYC, :])
        nc.sync.dma_start(
            out=_ap(self.y, 0, [(488, self.NYC), (1, 488)]), in_=ysb[:]
        )


_NC_CACHE = {}


def _get_nc():
    if "nc" not in _NC_CACHE:
        _NC_CACHE["nc"] = build_nc()
    return _NC_CACHE["nc"]


def _in_maps(inputs):
    x = np.ascontiguousarray(np.asarray(inputs["x"], dtype=np.float32))
    names = ["W1", "b1", "W2", "b2", "W3", "b3"]
    ws = {n: np.ascontiguousarray(np.asarray(inputs[n], np.float32)) for n in names}
    maps = []
    for i in range(8):
        m = {"x": x[i : i + 1]}
        m.update(ws)
        maps.append(m)
    return maps


def kernel(**inputs):
    nc = _get_nc()
    res = run_bass_kernel_spmd(nc, _in_maps(inputs), list(range(8)))
    return np.concatenate([res.results[i]["y"] for i in range(8)], axis=0)


def profile(**inputs):
    nc = _get_nc()
    res = run_bass_kernel_spmd(nc, _in_maps(inputs), list(range(8)), trace=True)
    return res.exec_time_ns


if __name__ == "__main__":
    rng = np.random.RandomState(0)
    ins = {
        "x": rng.randn(8, 1, H, W).astype(np.float32),
        "W1": rng.randn(CH, 1, 3, 3).astype(np.float32) * 0.1,
        "b1": np.zeros(CH, np.float32),
        "W2": rng.randn(CH, CH, 3, 3).astype(np.float32) * 0.05,
        "b2": np.zeros(CH, np.float32),
        "W3": rng.randn(C3, CH, 3, 3).astype(np.float32) * 0.05,
        "b3": np.zeros(C3, np.float32),
    }
    print(kernel(**ins).shape)
